# revision 19
# baseline (speedup 1.0000x reference)
"""DeepLSTM Trainium2 kernel.

Strategy (data-parallel over batch, 8 cores, no collectives):
  - Host: embedding gather, weight re-tiling to bf16 PE layout, batch
    sharding (8 rows/core), final logits GEMM + length-indexed capture.
  - Device, per core (identical SPMD program):
      Phase A: G0 = X @ Wx0 + b0 for all timesteps (one big GEMM).
      Phase B: FUSED two-layer LSTM recurrence. Per step: layer-0
               recurrent matmuls (Wh0), layer-1 recurrent matmuls (Wh1),
               and layer-1's input-to-gates matmuls (Wx1 applied to the
               y0 just produced this step) — 192 LDW+MM pairs of
               continuous PE work per step, under which both layers'
               cell-math chains (ACT sigmoid/tanh + DVE muls) hide
               completely. Gates are computed transposed
               [gate-dim in partitions, batch in free] so the cell math
               runs on wide [128, 16..64] tiles.
  - Host: logits = y1[b, len_b-1] @ W_out + b_out.

Masking in the reference is irrelevant to the output: logits only read
h2 at t = len-1, and every value feeding that is an unmasked h_new.

Device DRAM layouts (partition dim outermost, pure-slice DMAs):
  xt, y1: [128, T_pad, 4, 8]      (p, t, hidden-slice s, batch b)
  g0:     [128, 16, T_pad+2S, 8]  (p, perm gate-chunk j, t, b) — the
          per-(j, t-chunk) GEMM writeback is one contiguous run.
"""

import math
import sys

sys.path.insert(0, "/opt/trn_rl_repo")

import numpy as np
import ml_dtypes

HID = 512
NB = 8  # batch rows per core
NCORES = 8

# Gate-chunk permutation. Gate order in W columns is i,j,f,o (each 4 chunks
# of 128). Half-A covers hidden slices {0,1}, half-B {2,3}; within each half
# the order is i,i,f,f,o,o,j,j so sigmoid covers cols 0:48 and tanh 48:64 of
# the 64-wide per-half gate tile.
PERM = [0, 1, 8, 9, 12, 13, 4, 5, 2, 3, 10, 11, 14, 15, 6, 7]

BF16 = ml_dtypes.bfloat16

_PROG_CACHE = {}


def _build_program(T_pad, S):
    import concourse.bass as bass  # noqa: F401
    import concourse.tile as tile
    import concourse.bacc as bacc
    from concourse import mybir
    from concourse.bass import ds
    from contextlib import ExitStack
    from bass_rust import add_dep_helper

    fp32 = mybir.dt.float32
    bf16 = mybir.dt.bfloat16
    AF = mybir.ActivationFunctionType
    ET = mybir.EngineType

    n_body = 2 * S
    assert T_pad % n_body == 0
    n_iters = T_pad // n_body

    nc = bacc.Bacc("TRN2", target_bir_lowering=False, debug=False)

    xt = nc.dram_tensor("xt", [128, T_pad, 4, NB], bf16, kind="ExternalInput").ap()
    w0x = nc.dram_tensor("w0x", [128, 8192], bf16, kind="ExternalInput").ap()
    w0h = nc.dram_tensor("w0h", [128, 8192], bf16, kind="ExternalInput").ap()
    w1x = nc.dram_tensor("w1x", [128, 8192], bf16, kind="ExternalInput").ap()
    w1h = nc.dram_tensor("w1h", [128, 8192], bf16, kind="ExternalInput").ap()
    bg0 = nc.dram_tensor("bg0", [128, 16], fp32, kind="ExternalInput").ap()
    # b1 pre-broadcast to the transposed-gates tile: col hi*64+jj*8+b.
    bg1r = nc.dram_tensor("bg1r", [128, 128], fp32, kind="ExternalInput").ap()
    y1 = nc.dram_tensor("y1", [128, T_pad, 4, NB], fp32, kind="ExternalOutput").ap()

    def gemm_phase(tc, ctx, name, w_dram, b_dram, rhs_dram, g_dram):
        """g[:, j, t, b] = (x W + b) transposed, for all t; + zero pad rows."""
        nc = tc.nc
        wp = ctx.enter_context(tc.tile_pool(name=f"{name}w", bufs=1))
        w_sb = wp.tile([128, 8192], bf16, tag="w")
        nc.sync.dma_start(out=w_sb[:], in_=w_dram)
        b_sb = wp.tile([128, 16], fp32, tag="b")
        nc.sync.dma_start(out=b_sb[:], in_=b_dram)
        xp = ctx.enter_context(tc.tile_pool(name=f"{name}x", bufs=3))
        pp = ctx.enter_context(tc.tile_pool(name=f"{name}p", bufs=3, space="PSUM"))
        op = ctx.enter_context(tc.tile_pool(name=f"{name}o", bufs=4))

        for t0 in range(0, T_pad, 64):
            tcnt = min(64, T_pad - t0)
            ncols = NB * tcnt
            x_sb = xp.tile([128, 64, 4, NB], bf16, tag="x")
            nc.sync.dma_start(out=x_sb[:, :tcnt], in_=rhs_dram[:, t0 : t0 + tcnt])
            for j in range(16):
                ps = pp.tile([128, 512], fp32, tag="ps")
                for k in range(4):
                    nc.tensor.matmul(
                        ps[:, :ncols],
                        w_sb[:, (k * 16 + j) * 128 : (k * 16 + j + 1) * 128],
                        x_sb[:, :tcnt, k, :],
                        start=(k == 0),
                        stop=(k == 3),
                    )
                o_sb = op.tile([128, 512], fp32, tag="o")
                nc.scalar.activation(
                    o_sb[:, :ncols], ps[:, :ncols], AF.Identity, bias=b_sb[:, j : j + 1]
                )
                nc.sync.dma_start(
                    out=g_dram[:, j, t0 : t0 + tcnt, :], in_=o_sb[:, :ncols]
                )
        # Zero the 2S prefetch-overrun pad rows.
        zt = op.tile([128, 16, S, NB], fp32, tag="z")
        nc.vector.memset(zt[:], 0.0)
        for pi in range(2):
            r0 = T_pad + pi * S
            nc.sync.dma_start(out=g_dram[:, :, r0 : r0 + S, :], in_=zt[:])

    def fused_recur(tc, ctx, g_dram):
        nc = tc.nc
        wp = ctx.enter_context(tc.tile_pool(name="rw", bufs=1))
        wh0_sb = wp.tile([128, 8192], bf16, tag="wh0")
        nc.sync.dma_start(out=wh0_sb[:], in_=w0h)
        wh1_sb = wp.tile([128, 8192], bf16, tag="wh1")
        nc.sync.dma_start(out=wh1_sb[:], in_=w1h)
        wx1_sb = wp.tile([128, 8192], bf16, tag="wx1")
        nc.sync.dma_start(out=wx1_sb[:], in_=w1x)
        b1_sb = wp.tile([128, 128], fp32, tag="b1r")
        nc.sync.dma_start(out=b1_sb[:], in_=bg1r)

        st = ctx.enter_context(tc.tile_pool(name="rs", bufs=1))
        # y0 ring buffer: layer-0 h state AND layer-1 input, bf16, never DMA'd.
        ybE = st.tile([128, S, 4, NB], bf16, tag="ybE")
        ybO = st.tile([128, S, 4, NB], bf16, tag="ybO")
        # y1 output staging (fp32), DMA'd out per chunk.
        zbE = st.tile([128, S, 4, NB], fp32, tag="zbE")
        zbO = st.tile([128, S, 4, NB], fp32, tag="zbO")
        gE = st.tile([128, 16, S, NB], fp32, tag="gE")
        gO = st.tile([128, 16, S, NB], fp32, tag="gO")
        c0 = [
            [st.tile([128, 16], fp32, tag=f"c0{h}{i}", name=f"c0{h}{i}") for i in (0, 1)]
            for h in (0, 1)
        ]
        c1 = [
            [st.tile([128, 16], fp32, tag=f"c1{h}{i}", name=f"c1{h}{i}") for i in (0, 1)]
            for h in (0, 1)
        ]
        h1 = [
            [st.tile([128, 16], bf16, tag=f"h1{h}{i}", name=f"h1{h}{i}") for i in (0, 1)]
            for h in (0, 1)
        ]

        pp = ctx.enter_context(tc.tile_pool(name="rp", bufs=2, space="PSUM"))
        ep = ctx.enter_context(tc.tile_pool(name="re", bufs=3))

        # Prologue: zero state, load chunk 0.
        for h in (0, 1):
            nc.vector.memset(c0[h][0][:], 0.0)
            nc.vector.memset(c1[h][0][:], 0.0)
            nc.vector.memset(h1[h][0][:], 0.0)
        nc.vector.memset(ybO[:, S - 1], 0.0)
        nc.sync.dma_start(out=gE[:], in_=g_dram[:, :, 0:S, :])

        def mm_block(ps, w_sb, jbase, ks, rhs4, start, stop):
            """16 LDW+MM pairs: gate chunks jbase..jbase+8 x contraction
            chunks ks, rhs4[k] the [128, NB] moving operand per k-chunk."""
            out = []
            for jj in range(8):
                j = jbase + jj
                for k in ks:
                    out.append(
                        nc.tensor.matmul(
                            ps[:, jj * 8 : jj * 8 + 8],
                            w_sb[:, (k * 16 + j) * 128 : (k * 16 + j + 1) * 128],
                            rhs4[k],
                            start=(start and jj == 0 and k == ks[0]),
                            stop=(stop and jj == 7 and k == ks[1]),
                        )
                    )
            return out

        def cell_chain(hi, ps, gadd, c_pair, par, y_out, h_out, tagp):
            """LSTM cell math for one gate half: gates=[i,i,f,f,o,o,j,j]x8b."""
            gt = ep.tile([128, 64], fp32, tag=f"gt{tagp}{hi}", name=f"gt{tagp}{hi}")
            nc.vector.tensor_add(gt[:], ps[:], gadd)
            sg = ep.tile([128, 48], fp32, tag=f"sg{tagp}{hi}", name=f"sg{tagp}{hi}")
            nc.scalar.activation(sg[:], gt[:, 0:48], AF.Sigmoid)
            tj = ep.tile([128, 16], fp32, tag=f"tj{tagp}{hi}", name=f"tj{tagp}{hi}")
            nc.scalar.activation(tj[:], gt[:, 48:64], AF.Tanh)
            m1 = ep.tile([128, 16], fp32, tag=f"m1{tagp}{hi}", name=f"m1{tagp}{hi}")
            nc.vector.tensor_mul(m1[:], sg[:, 0:16], tj[:])
            m2 = ep.tile([128, 16], fp32, tag=f"m2{tagp}{hi}", name=f"m2{tagp}{hi}")
            c_prev = c_pair[par]
            c_new = c_pair[1 - par]
            nc.vector.tensor_mul(m2[:], sg[:, 16:32], c_prev[:])
            nc.vector.tensor_add(c_new[:], m1[:], m2[:])
            tch = ep.tile([128, 16], fp32, tag=f"tc{tagp}{hi}", name=f"tc{tagp}{hi}")
            nc.scalar.activation(tch[:], c_new[:], AF.Tanh)
            nc.vector.tensor_mul(y_out, sg[:, 32:48], tch[:])
            if h_out is not None:
                nc.vector.tensor_copy(h_out, y_out)

        with tc.For_i(
            0,
            n_iters * n_body,
            n_body,
            hint_engines=(ET.PE, ET.DVE, ET.Activation),
            name="recloop",
        ) as i:
            for hc, (ybuf, ybuf_prev, zbuf, gbuf) in enumerate(
                [(ybE, ybO, zbE, gE), (ybO, ybE, zbO, gO)]
            ):
                if hc == 0:
                    nc.sync.dma_start(out=gO[:], in_=g_dram[:, :, ds(i + S, S), :])
                for dt in range(S):
                    step = hc * S + dt
                    par = step % 2
                    if dt == 0:
                        h0src = [ybuf_prev[:, S - 1, s, :] for s in range(4)]
                    else:
                        h0src = [ybuf[:, dt - 1, s, :] for s in range(4)]
                    h1src = [
                        h1[s // 2][par][:, (s % 2) * 8 : (s % 2) * 8 + 8]
                        for s in range(4)
                    ]
                    y0src = [ybuf[:, dt, s, :] for s in range(4)]

                    ps0A = pp.tile([128, 64], fp32, tag="ps0A", name="ps0A")
                    ps0B = pp.tile([128, 64], fp32, tag="ps0B", name="ps0B")
                    ps1A = pp.tile([128, 64], fp32, tag="ps1A", name="ps1A")
                    ps1B = pp.tile([128, 64], fp32, tag="ps1B", name="ps1B")

                    blocks = []
                    # Layer-0 recurrence: k01 (h half-A) first, then k23.
                    for ks in ((0, 1), (2, 3)):
                        for jb, ps in ((0, ps0A), (8, ps0B)):
                            blocks.append(
                                mm_block(
                                    ps, wh0_sb, jb, ks, h0src,
                                    start=(ks[0] == 0), stop=(ks[1] == 3),
                                )
                            )
                    # Layer-0 cell math -> y0 (bf16, into the ring buffer).
                    # Emitted before the layer-1 input matmuls that read y0
                    # (Tile binds dependencies in trace order).
                    for hi, ps in ((0, ps0A), (1, ps0B)):
                        cell_chain(
                            hi, ps,
                            gbuf[:, hi * 8 : hi * 8 + 8, dt, :],
                            c0[hi], par,
                            ybuf[:, dt, 2 * hi : 2 * hi + 2, :],
                            None, "a",
                        )
                    # Layer-1 recurrence into ps1 (group start).
                    for ks in ((0, 1), (2, 3)):
                        for jb, ps in ((0, ps1A), (8, ps1B)):
                            blocks.append(
                                mm_block(
                                    ps, wh1_sb, jb, ks, h1src,
                                    start=(ks[0] == 0), stop=False,
                                )
                            )
                    # Layer-1 input part from this step's y0 (group stop).
                    for ks in ((0, 1), (2, 3)):
                        for jb, ps in ((0, ps1A), (8, ps1B)):
                            blocks.append(
                                mm_block(
                                    ps, wx1_sb, jb, ks, y0src,
                                    start=False, stop=(ks[1] == 3),
                                )
                            )
                    for prev_b, next_b in zip(blocks, blocks[1:]):
                        add_dep_helper(
                            next_b[0].ins,
                            prev_b[-1].ins,
                            sync=True,
                            reason="pin per-step PE block order",
                        )
                    # Layer-1 cell math -> h1 (bf16) + y1 staging (fp32).
                    for hi, ps in ((0, ps1A), (1, ps1B)):
                        cell_chain(
                            hi, ps,
                            b1_sb[:, hi * 64 : hi * 64 + 64],
                            c1[hi], par,
                            h1[hi][1 - par][:],
                            zbuf[:, dt, 2 * hi : 2 * hi + 2, :],
                            "b",
                        )
                # y1 chunk writeback
                nc.sync.dma_start(out=y1[:, ds(i + hc * S, S)], in_=zbuf[:])
                if hc == 0:
                    nc.sync.dma_start(
                        out=gE[:], in_=g_dram[:, :, ds(i + 2 * S, S), :]
                    )

    with ExitStack() as ctx:
        tc = ctx.enter_context(tile.TileContext(nc))
        dram = ctx.enter_context(tc.tile_pool(name="dram", bufs=1, space="DRAM"))
        g0 = dram.tile([128, 16, T_pad + 2 * S, NB], fp32, tag="g0")

        with ExitStack() as c1:
            gemm_phase(tc, c1, "ga", w0x, bg0, xt, g0)
        tc.strict_bb_all_engine_barrier()
        with ExitStack() as c2:
            fused_recur(tc, c2, g0)

    nc.compile()
    return nc


def _prep_w_half(Wp):
    """[512, 2048] f32 -> [128, 4*16*128] bf16 in PE lhsT tile layout."""
    arr = Wp.reshape(4, 128, 16, 128)[:, :, PERM, :]
    return np.ascontiguousarray(
        np.transpose(arr, (1, 0, 2, 3)).reshape(128, 8192)
    ).astype(BF16)


def _prep_b(b):
    """[2048] f32 -> [128, 16] f32, col j = b[PERM[j]*128 : +128]."""
    return np.ascontiguousarray(b.reshape(16, 128)[PERM].T).astype(np.float32)


def build_in_maps(inputs_np, T_pad):
    ids = np.asarray(inputs_np["inputs"]).astype(np.int64)
    emb = np.asarray(inputs_np["emb"], dtype=np.float32)
    W0 = np.asarray(inputs_np["W0"], dtype=np.float32)
    b0 = np.asarray(inputs_np["b0"], dtype=np.float32)
    W1 = np.asarray(inputs_np["W1"], dtype=np.float32)
    b1 = np.asarray(inputs_np["b1"], dtype=np.float32)

    T_full = ids.shape[1]
    ids_p = ids[:, : min(T_pad, T_full)]
    X = emb[ids_p]  # [B, t, 512] f32
    if X.shape[1] < T_pad:
        X = np.concatenate(
            [X, np.zeros((X.shape[0], T_pad - X.shape[1], HID), np.float32)], axis=1
        )
    B = X.shape[0]
    assert B % NB == 0
    ncores = B // NB

    bg1 = _prep_b(b1)  # [128, 16]
    bg1r = np.repeat(bg1.reshape(128, 16, 1), NB, axis=2).reshape(128, 128)

    shared = {
        "w0x": _prep_w_half(W0[:512]),
        "w0h": _prep_w_half(W0[512:]),
        "w1x": _prep_w_half(W1[:512]),
        "w1h": _prep_w_half(W1[512:]),
        "bg0": _prep_b(b0),
        "bg1r": np.ascontiguousarray(bg1r).astype(np.float32),
    }
    in_maps = []
    for c in range(ncores):
        Xc = X[c * NB : (c + 1) * NB]  # [NB, T_pad, 512]
        # xt[p, t, s, b] = Xc[b, t, s*128+p]
        xtc = np.ascontiguousarray(
            np.transpose(Xc.reshape(NB, T_pad, 4, 128), (3, 1, 2, 0))
        ).astype(BF16)
        in_maps.append({"xt": xtc, **shared})
    return in_maps


def finish_output(results, inputs_np, T_pad):
    lens = np.asarray(inputs_np["input_length"]).astype(np.int64)
    W_out = np.asarray(inputs_np["W_out"], dtype=np.float32)
    b_out = np.asarray(inputs_np["b_out"], dtype=np.float32)
    B = lens.shape[0]
    last = np.empty((B, HID), np.float32)
    for c in range(B // NB):
        y1c = np.asarray(results[c]["y1"])  # [128, T_pad, 4, NB] f32
        for bb in range(NB):
            b = c * NB + bb
            t = int(lens[b]) - 1
            last[b] = y1c[:, t, :, bb].T.reshape(HID)  # hidden = s*128 + p
    return (last @ W_out + b_out).astype(np.float32)


def kernel(**inputs):
    from concourse import bass_utils

    inputs_np = {k: np.asarray(v) for k, v in inputs.items()}
    lens = inputs_np["input_length"].astype(np.int64)
    S = 50
    max_len = int(lens.max())
    T_pad = max(2 * S, int(math.ceil(max_len / (2 * S))) * 2 * S)

    key = (T_pad, S)
    if key not in _PROG_CACHE:
        _PROG_CACHE[key] = _build_program(T_pad, S)
    nc = _PROG_CACHE[key]

    in_maps = build_in_maps(inputs_np, T_pad)
    res = bass_utils.run_bass_kernel_spmd(
        nc, in_maps, core_ids=list(range(len(in_maps)))
    )
    return finish_output(res.results, inputs_np, T_pad)
